# revision 19
# baseline (speedup 1.0000x reference)
"""Causal self-attention (B=2, S=2048, D=2048, H=16) on 8 TRN2 NeuronCores.

Sharding: 2 batches x 4 head-groups.  Core c handles batch c//4 and heads
[4*(c%4) .. 4*(c%4)+3]; each core produces output columns [512*(c%4) ...].

v4 schedule: q/k weights ship as 16 [128, 1024] panels (2KB DMA rows; the
head-0 column range rides its own first DMA so the first projection chain
starts ~6us in).  The softmax reciprocal broadcast runs as a K=1
ones-matmul on the PE (GpSimd carries only the AllGather doorbells, which
head-of-line block its FIFO for a full AG).  Heads 0-2 AllGather their
normalized yT in token halves (descending q-blocks); head 3 walks its
q-blocks ASCENDING and gathers per token quarter, so gathers drain in the
same order the out projection consumes them.  The out projection splits
each token tile into an h0-2 chain (12 matmuls, parked as bf16 partials in
SBUF slots the xT pool no longer needs -- this is the PE filler while
head-3's ACT-paced attention runs, keeping HAM warm) and an h3 chain (4
matmuls) ordered after the last attention matmul.

Softmax uses exp without max subtraction (logits are O(8) here);
denominators are accumulated on DVE over k-tile pairs, reduced across
partitions with a ones-matmul, inverted with reciprocal_approx_fast, and
broadcast back with a K=1 ones-matmul, staged through SBUF on ScalarE
(DVE cannot read two PSUM operands).

Compute is bf16 with fp32 PSUM accumulation; measured l2 rel err vs the
fp32 reference is ~5.8e-3.
"""

import numpy as np
import ml_dtypes

B, S, D = 2, 2048, 2048
H, HD = 16, 128
HLOC = 4           # heads per core
CW = HLOC * HD     # 512: per-core v width and out-column width
QB = 4             # q blocks of 512
DT = 16            # d tiles of 128
TB = 4             # token blocks of 512
SCALE = 1.0 / float(np.sqrt(HD))
GROUPS = [[0, 1, 2, 3], [4, 5, 6, 7]]
HLAST = HLOC - 1

_cache = {}


def _build():
    import concourse.tile as tile
    import concourse.mybir as mybir
    from concourse import bacc
    from concourse.tile import add_dep_helper

    BF = mybir.dt.bfloat16
    F32 = mybir.dt.float32

    nc = bacc.Bacc("TRN2", target_bir_lowering=False, debug=False, num_devices=8)

    # Inputs (per-core shards, host-prepared)
    xT = nc.dram_tensor("xT", [D, S], BF, kind="ExternalInput")          # x[batch].T
    # wqk[dt][:, (2h+qk)*128 : +128] = w_qkv d-tile dt for local head h
    wqk = nc.dram_tensor("wqk", [DT, 128, 2 * HLOC * 128], BF,
                         kind="ExternalInput")
    wv = nc.dram_tensor("wv", [DT, 128, CW], BF, kind="ExternalInput")
    bqk = nc.dram_tensor("bqk", [128, 2 * HLOC], F32, kind="ExternalInput")
    bv = nc.dram_tensor("bv", [1, CW], F32, kind="ExternalInput")
    # w_out rows permuted: wout[h][i] = w_out[512*i + 128*h : +128, cols]
    wout = nc.dram_tensor("wout", [HLOC, 4, 128, CW], BF, kind="ExternalInput")
    bout = nc.dram_tensor("bout", [1, CW], F32, kind="ExternalInput")
    out = nc.dram_tensor("out", [S, CW], BF, kind="ExternalOutput")

    # AG buffers: heads 0-2 per token half; head 3 per token quarter
    # (q-block), so the last head's gathers land before its attention ends.
    ag_in = {(h, hf): nc.dram_tensor(f"ag_in{h}_{hf}", [128, 1024], BF,
                                     kind="Internal")
             for h in range(HLAST) for hf in range(2)}
    ag_out = {(h, hf): nc.dram_tensor(f"ag_out{h}_{hf}", [512, 1024], BF,
                                      kind="Internal")
              for h in range(HLAST) for hf in range(2)}
    ag3_in = {qb: nc.dram_tensor(f"ag3_in{qb}", [128, 512], BF, kind="Internal")
              for qb in range(QB)}
    ag3_out = {qb: nc.dram_tensor(f"ag3_out{qb}", [512, 512], BF,
                                  kind="Internal")
               for qb in range(QB)}

    last_attention_mm = [None]   # last PE instruction of head-3 attention
    last_agin_dma = [None]       # last ag_in write on the sync DMA queue

    with tile.TileContext(nc) as tc:
        with (
            tc.tile_pool(name="const", bufs=1) as constp,
            tc.tile_pool(name="pers", bufs=1) as pers,
            tc.tile_pool(name="work", bufs=2) as work,
            tc.tile_pool(name="psum", bufs=2, space="PSUM") as psum,
        ):
            # ---- packed q/k biases: one [128, 8] tile, column h*2+qk ----
            bqk_sb = constp.tile([128, 2 * HLOC], F32, name="bqk_sb")
            nc.sync.dma_start(bqk_sb[:], bqk[:])

            # ---- q/k weight panels: [128, 1024] per d-tile, all heads.
            # Head 0's 256 columns ride a separate first DMA so the first
            # projection chain is not gated on the full panel. ----
            wqk_sb = []
            for dt in range(DT):
                t = constp.tile([128, 2 * HLOC * 128], BF, name=f"wqk{dt}")
                nc.sync.dma_start(t[:, 0:256], wqk[dt, :, 0:256])
                wqk_sb.append(t)

            # xT ships as [128, 1024] half-rows (half the DMA packets of
            # [128, 512] tiles); token half 0 first, then wv, then half 1
            xt_rows = {}
            wv_sb = [None] * DT

            def load_xt(dt, th):
                t = work.tile([128, 1024], BF, name=f"xt_{dt}_{th}",
                              tag="xT", bufs=32)
                nc.sync.dma_start(
                    t[:], xT[dt * 128:(dt + 1) * 128,
                             th * 1024:(th + 1) * 1024])
                xt_rows[(dt, th)] = t

            def xt_ap(dt, tb, lo=0, n=512):
                base = (tb % 2) * 512 + lo
                return xt_rows[(dt, tb // 2)][:, base:base + n]

            for dt in range(DT):
                load_xt(dt, 0)
            for dt in range(DT):
                wvp = work.tile([128, CW], BF, name=f"wvp{dt}", tag="p512",
                                bufs=17)
                nc.sync.dma_start(wvp[:], wv[dt])
                wv_sb[dt] = wvp
            # heads 1-3 weight columns
            for dt in range(DT):
                nc.sync.dma_start(wqk_sb[dt][:, 256:1024],
                                  wqk[dt, :, 256:1024])
            for dt in range(DT):
                load_xt(dt, 1)

            # ---- per-head q/k projection ([col, tok] transposed) ----
            def qk_proj(h):
                dests = {}
                for qk in range(2):
                    c = (2 * h + qk) * 128
                    dest = work.tile([128, S], BF, name=f"qkT_{h}_{qk}",
                                     tag="qkT", bufs=4)
                    for tb in range(TB):
                        acc = psum.tile([128, 512], F32, name="acc_qk",
                                        tag="acc", bufs=2)
                        for dt in range(DT):
                            nc.tensor.matmul(
                                acc[:], wqk_sb[dt][:, c:c + 128],
                                xt_ap(dt, tb),
                                start=(dt == 0), stop=(dt == DT - 1),
                            )
                        cb = 2 * h + qk
                        nc.scalar.activation(
                            dest[:, tb * 512:(tb + 1) * 512], acc[:],
                            mybir.ActivationFunctionType.Identity,
                            bias=bqk_sb[:, cb:cb + 1], scale=1.0,
                        )
                    dests[qk] = dest
                return dests[0], dests[1]

            qk_tiles = qk_proj(0)

            # ---- constants ----
            ones = constp.tile([128, 1], BF, name="ones")
            nc.gpsimd.memset(ones[:], 1.0)
            ones1 = constp.tile([1, 128], BF, name="ones1")
            nc.gpsimd.memset(ones1[:], 1.0)

            # Pair masks for the 4 diagonal k-subtiles, packed two subtiles
            # wide: pairmask[m][:, 512*sub + qq] keeps where
            # qq >= kk + 128*(2m+sub).
            pairmasks = []
            for pm in range(2):
                m = constp.tile([128, 1024], BF, name=f"pmask{pm}",
                                tag=f"pmask{pm}")
                nc.gpsimd.memset(m[:], 1.0)
                for sub in range(2):
                    nc.gpsimd.affine_select(
                        out=m[:, sub * 512:(sub + 1) * 512],
                        in_=m[:, sub * 512:(sub + 1) * 512],
                        compare_op=mybir.AluOpType.is_ge, fill=0.0,
                        base=-128 * (2 * pm + sub), channel_multiplier=-1,
                        pattern=[[1, 512]],
                    )
                pairmasks.append(m)

            bout_sb = constp.tile([1, CW], F32, name="bout_sb")
            nc.sync.dma_start(bout_sb[:], bout[:])
            bias_bc = constp.tile([128, CW], F32, name="bias_bc")
            nc.gpsimd.partition_broadcast(bias_bc[:], bout_sb[:], channels=128)

            bv_sb = constp.tile([1, CW], F32, name="bv_sb")
            nc.sync.dma_start(bv_sb[:], bv[:])
            vbias_bc = constp.tile([128, CW], F32, name="vbias_bc")
            nc.gpsimd.partition_broadcast(vbias_bc[:], bv_sb[:], channels=128)

            # ---- persistent v tiles ----
            vt = [pers.tile([128, CW], BF, name=f"v{t}", tag=f"v{t}")
                  for t in range(16)]

            # ---- v projection: v[t] = x @ wv  ([tok, vcol], xT stationary) ----
            for t in range(16):
                tb, j = t // 4, t % 4
                acc = psum.tile([128, CW], F32, name="acc_v", tag="acc", bufs=2)
                for dt in range(DT):
                    nc.tensor.matmul(
                        acc[:],
                        xt_ap(dt, tb, j * 128, 128),
                        wv_sb[dt][:],
                        start=(dt == 0), stop=(dt == DT - 1),
                    )
                nc.vector.tensor_tensor(vt[t][:], acc[:], vbias_bc[:],
                                        mybir.AluOpType.add)

            # ---- attention for one head + its AGs.  Heads 0-2 descend
            # through q-blocks (token-half 1 gathered first); head 3
            # ascends and gathers per q-block. ----
            def attention_head(h, qTh, kTh, post_qb=None):
                qb_order = (0, 1, 2, 3) if h == HLAST else (3, 2, 1, 0)
                for qb in qb_order:
                    nk = 4 * qb + 4
                    y_ps = psum.tile([128, 512], F32, name="y_ps", tag="y")
                    esum = work.tile([128, 1024], BF, name="esum", tag="esum",
                                     bufs=2)
                    prev = None

                    def flush(prev_pair):
                        e, pr = prev_pair
                        for s_ in range(2):
                            kt = 2 * pr + s_
                            nc.tensor.matmul(
                                y_ps[:],
                                vt[kt][:, h * 128:(h + 1) * 128],
                                e[:, s_ * 512:(s_ + 1) * 512],
                                start=(kt == 0), stop=(kt == nk - 1),
                            )
                        if pr == 0:
                            nc.vector.tensor_copy(esum[:], e[:])
                        else:
                            nc.vector.tensor_tensor(esum[:], esum[:], e[:],
                                                    mybir.AluOpType.add)

                    for pr in range(nk // 2):
                        sc = psum.tile([128, 1024], F32, name="sc", tag="s",
                                       bufs=2)
                        for s_ in range(2):
                            kt = 2 * pr + s_
                            nc.tensor.matmul(
                                sc[:, s_ * 512:(s_ + 1) * 512],
                                kTh[:, kt * 128:(kt + 1) * 128],
                                qTh[:, qb * 512:(qb + 1) * 512],
                                start=True, stop=True,
                            )
                        e = work.tile([128, 1024], BF, name="expT", tag="expT",
                                      bufs=3)
                        nc.scalar.activation(
                            e[:], sc[:], mybir.ActivationFunctionType.Exp,
                            scale=SCALE,
                        )
                        pm = pr - (nk // 2 - 2)
                        if pm >= 0:
                            nc.vector.tensor_tensor(e[:], e[:],
                                                    pairmasks[pm][:],
                                                    mybir.AluOpType.mult)
                        if prev is not None:
                            flush(prev)
                        prev = (e, pr)
                    flush(prev)

                    esum_f = work.tile([128, 512], BF, name="esum_f",
                                       tag="esum_f", bufs=2)
                    nc.vector.tensor_tensor(esum_f[:], esum[:, 0:512],
                                            esum[:, 512:1024],
                                            mybir.AluOpType.add)
                    # nb: one PSUM bank serving the partition-sum (row 0)
                    # and then the broadcast of its reciprocal (all rows).
                    nb = psum.tile([128, 512], F32, name="nb", tag="y")
                    nc.tensor.matmul(nb[0:1, :], ones[:], esum_f[:],
                                     start=True, stop=True)
                    recip = work.tile([1, 512], F32, name="recip", tag="recip",
                                      bufs=2)
                    nc.vector.reciprocal_approx_fast(recip[:], nb[0:1, :])
                    recip_b = work.tile([1, 512], BF, name="recip_b",
                                        tag="recip_b", bufs=2)
                    nc.vector.tensor_copy(recip_b[:], recip[:])
                    mm_bc = nc.tensor.matmul(nb[:, :], ones1[:], recip_b[:],
                                             start=True, stop=True)
                    if h == HLAST:
                        last_attention_mm[0] = mm_bc
                    # DVE can't read two PSUM operands; stage the broadcast
                    # through SBUF on the (otherwise idle) scalar engine.
                    nbs = work.tile([128, 512], BF, name="nbs", tag="nbs",
                                    bufs=2)
                    nc.scalar.activation(
                        nbs[:], nb[:], mybir.ActivationFunctionType.Identity,
                        scale=1.0,
                    )
                    ynorm = work.tile([128, 512], BF, name="ynorm", tag="ynorm",
                                      bufs=3)
                    nc.vector.tensor_tensor(ynorm[:], y_ps[:], nbs[:],
                                            mybir.AluOpType.mult)
                    if h == HLAST:
                        d = nc.sync.dma_start(ag3_in[qb][:], ynorm[:])
                        last_agin_dma[0] = d
                        nc.gpsimd.collective_compute(
                            "AllGather", mybir.AluOpType.bypass,
                            replica_groups=GROUPS,
                            ins=[ag3_in[qb].ap()],
                            outs=[ag3_out[qb].ap()],
                        )
                        if post_qb is not None:
                            post_qb(qb)
                    else:
                        hf, co = qb // 2, (qb % 2) * 512
                        nc.sync.dma_start(
                            ag_in[(h, hf)][:, co:co + 512], ynorm[:])
                        if qb in (2, 0):
                            nc.gpsimd.collective_compute(
                                "AllGather", mybir.AluOpType.bypass,
                                replica_groups=GROUPS,
                                ins=[ag_in[(h, hf)].ap()],
                                outs=[ag_out[(h, hf)].ap()],
                            )

            # ---- head pipeline.  During head 3's attention, each q-block
            # hook issues the ygt readbacks for one out-projection block
            # (their AG waits are already satisfied, so the sync DMA queue
            # never head-of-line blocks between head-3's ag writes). ----
            wout_sb = {}
            ygt = {}
            ygt3 = {}

            def load_wout():
                for h in range(HLOC):
                    for i in range(4):
                        t = work.tile([128, CW], BF, name=f"wout{h}{i}",
                                      tag="p512", bufs=17)
                        nc.sync.dma_start(t[:], wout[h, i])
                        wout_sb[(h, i)] = t

            def load_ygt(tc_):
                hf, co = tc_ // 2, (tc_ % 2) * 512
                for h in range(HLAST):
                    for i in range(4):
                        t = work.tile([128, 512], BF, name=f"yg_{h}_{tc_}_{i}",
                                      tag="ygt", bufs=16)
                        nc.sync.dma_start(
                            t[:], ag_out[(h, hf)][i * 128:(i + 1) * 128,
                                                  co:co + 512])
                        ygt[(h, tc_, i)] = t

            def load_ygt3():
                for tc_ in (0, 1, 2, 3):
                    for i in range(4):
                        t3 = work.tile([128, 512], BF, name=f"yg3_{tc_}_{i}",
                                       tag="ygt3", bufs=6)
                        nc.sync.dma_start(
                            t3[:], ag3_out[tc_][i * 128:(i + 1) * 128, :])
                        ygt3[(tc_, i)] = t3

            def att3_hook(qb):
                load_ygt({0: 2, 1: 3, 2: 0, 3: 1}[qb])
                if qb == 3:
                    load_ygt3()

            for h in range(HLOC):
                if h == HLAST:
                    attention_head(h, *qk_tiles, post_qb=att3_hook)
                else:
                    attention_head(h, *qk_tiles)
                    qk_tiles = qk_proj(h + 1)
                    if h == 0:
                        load_wout()

            # ---- out projection, split per token tile:
            #   chain A: heads 0-2 (12 matmuls) -> bf16 partial (+bias) in
            #            SBUF slots the xT pool no longer needs.  This is
            #            the PE filler during head-3's attention.
            #   chain B: head 3 (4 matmuls), ordered after the last
            #            attention matmul, + final combine and store. ----
            first_b_mm = [None]
            partA = {}
            for tc_ in (2, 3, 0, 1):
                for j in range(4):
                    t = tc_ * 4 + j
                    accA = psum.tile([128, CW], F32, name="acc_a",
                                     tag="acc", bufs=2)
                    nmm = 0
                    for h in range(HLAST):
                        for i in range(4):
                            nc.tensor.matmul(
                                accA[:],
                                ygt[(h, tc_, i)][:, j * 128:(j + 1) * 128],
                                wout_sb[(h, i)][:],
                                start=(nmm == 0), stop=(nmm == 11),
                            )
                            nmm += 1
                    if t % 2 == 0:
                        partA[t // 2] = work.tile([128, 1024], BF,
                                                  name=f"partA{t // 2}",
                                                  tag="xT", bufs=32)
                    pa = partA[t // 2][:, (t % 2) * 512:(t % 2) * 512 + 512]
                    nc.vector.tensor_tensor(pa, accA[:], bias_bc[:],
                                            mybir.AluOpType.add)
            for tc_ in (0, 1, 2, 3):
                for j in range(4):
                    t = tc_ * 4 + j
                    accB = psum.tile([128, CW], F32, name="acc_b",
                                     tag="y", bufs=2)
                    for i in range(4):
                        mm = nc.tensor.matmul(
                            accB[:],
                            ygt3[(tc_, i)][:, j * 128:(j + 1) * 128],
                            wout_sb[(HLAST, i)][:],
                            start=(i == 0), stop=(i == 3),
                        )
                        if first_b_mm[0] is None:
                            first_b_mm[0] = mm
                    osb = work.tile([128, CW], BF, name="osb",
                                    tag="osb", bufs=3)
                    pa = partA[t // 2][:, (t % 2) * 512:(t % 2) * 512 + 512]
                    nc.vector.tensor_tensor(osb[:], accB[:], pa,
                                            mybir.AluOpType.add)
                    nc.sync.dma_start(out[t * 128:t * 128 + 64, :],
                                      osb[0:64, :])
                    nc.sync.dma_start(out[t * 128 + 64:(t + 1) * 128, :],
                                      osb[64:128, :])

            add_dep_helper(first_b_mm[0].ins, last_attention_mm[0].ins,
                           sync=False,
                           reason="h3 outproj PE stream after last attention mm")

    nc.compile()
    return nc


def _prep_inputs(x, w_qkv, b_qkv, w_out, b_out):
    """Host-side sharding/layout. Returns in_maps for the 8 cores."""
    bf16 = ml_dtypes.bfloat16
    x = np.asarray(x, dtype=np.float32)
    w_qkv = np.asarray(w_qkv, dtype=np.float32)
    b_qkv = np.asarray(b_qkv, dtype=np.float32)
    w_out = np.asarray(w_out, dtype=np.float32)
    b_out = np.asarray(b_out, dtype=np.float32)

    xT_b = [np.ascontiguousarray(x[b].T).astype(bf16) for b in range(B)]

    in_maps = []
    for c in range(8):
        b, g = c // 4, c % 4
        cols = slice(CW * g, CW * (g + 1))

        # wqk[dt][:, (2h+qk)*128:+128] = d-tile dt of w_q/w_k for head 4g+h
        wqk = np.empty((DT, 128, 2 * HLOC * 128), np.float32)
        bqk = np.empty((128, 2 * HLOC), np.float32)
        for h in range(HLOC):
            gh = 4 * g + h
            for qk in range(2):
                wcol = w_qkv[:, qk * D + 128 * gh: qk * D + 128 * (gh + 1)]
                wqk[:, :, (2 * h + qk) * 128:(2 * h + qk + 1) * 128] = \
                    wcol.reshape(DT, 128, 128)
                bqk[:, 2 * h + qk] = b_qkv[qk * D + 128 * gh: qk * D + 128 * (gh + 1)]

        wv_ = w_qkv[:, 2 * D:3 * D][:, cols]
        bv_ = b_qkv[2 * D:3 * D][cols]

        # w_out rows permuted to the AG's rank-major order per head chunk
        wout_loc = w_out[:, cols]
        wout_t = np.empty((HLOC, 4, 128, CW), np.float32)
        for h in range(HLOC):
            for i in range(4):
                wout_t[h, i] = wout_loc[512 * i + 128 * h: 512 * i + 128 * (h + 1), :]

        in_maps.append({
            "xT": xT_b[b],
            "wqk": np.ascontiguousarray(wqk).astype(bf16),
            "wv": np.ascontiguousarray(wv_.reshape(DT, 128, CW)).astype(bf16),
            "bqk": np.ascontiguousarray(bqk),
            "bv": np.ascontiguousarray(bv_.reshape(1, CW)),
            "wout": np.ascontiguousarray(wout_t).astype(bf16),
            "bout": np.ascontiguousarray(b_out[cols].reshape(1, CW)),
        })
    return in_maps


def kernel(x, w_qkv, b_qkv, w_out, b_out, _trace=False, _trace_kwargs=None):
    from concourse.bass_utils import run_bass_kernel_spmd

    if "nc" not in _cache:
        _cache["nc"] = _build()
    nc = _cache["nc"]

    in_maps = _prep_inputs(x, w_qkv, b_qkv, w_out, b_out)
    res = run_bass_kernel_spmd(
        nc, in_maps, core_ids=list(range(8)),
        trace=_trace, **(_trace_kwargs or {}),
    )

    out = np.empty((B, S, D), dtype=np.float32)
    for c in range(8):
        b, g = c // 4, c % 4
        out[b][:, CW * g:CW * (g + 1)] = np.asarray(res.results[c]["out"], dtype=np.float32)
    kernel.last_result = res
    return out


# revision 20
# speedup vs baseline: 1.0259x; 1.0259x over previous
"""Causal self-attention (B=2, S=2048, D=2048, H=16) on 8 TRN2 NeuronCores.

Sharding: 2 batches x 4 head-groups.  Core c handles batch c//4 and heads
[4*(c%4) .. 4*(c%4)+3]; each core produces output columns [512*(c%4) ...].

v4 schedule: q/k weights ship as 16 [128, 1024] panels (2KB DMA rows; the
head-0 column range rides its own first DMA so the first projection chain
starts ~6us in).  The softmax reciprocal broadcast runs as a K=1
ones-matmul on the PE (GpSimd carries only the AllGather doorbells, which
head-of-line block its FIFO for a full AG).  Heads 0-2 AllGather their
normalized yT in token halves (descending q-blocks); head 3 walks its
q-blocks ASCENDING and gathers per token quarter, so gathers drain in the
same order the out projection consumes them.  The out projection splits
each token tile into an h0-2 chain (12 matmuls, parked as bf16 partials in
SBUF slots the xT pool no longer needs -- this is the PE filler while
head-3's ACT-paced attention runs, keeping HAM warm) and an h3 chain (4
matmuls) ordered after the last attention matmul.

Softmax uses exp without max subtraction (logits are O(8) here);
denominators are accumulated on DVE over k-tile pairs, reduced across
partitions with a ones-matmul, inverted with reciprocal_approx_fast, and
broadcast back with a K=1 ones-matmul, staged through SBUF on ScalarE
(DVE cannot read two PSUM operands).

Compute is bf16 with fp32 PSUM accumulation; measured l2 rel err vs the
fp32 reference is ~5.8e-3.
"""

import numpy as np
import ml_dtypes

B, S, D = 2, 2048, 2048
H, HD = 16, 128
HLOC = 4           # heads per core
CW = HLOC * HD     # 512: per-core v width and out-column width
QB = 4             # q blocks of 512
DT = 16            # d tiles of 128
TB = 4             # token blocks of 512
SCALE = 1.0 / float(np.sqrt(HD))
GROUPS = [[0, 1, 2, 3], [4, 5, 6, 7]]
HLAST = HLOC - 1

_cache = {}


def _build():
    import concourse.tile as tile
    import concourse.mybir as mybir
    from concourse import bacc
    from concourse.tile import add_dep_helper

    BF = mybir.dt.bfloat16
    F32 = mybir.dt.float32

    nc = bacc.Bacc("TRN2", target_bir_lowering=False, debug=False, num_devices=8)

    # Inputs (per-core shards, host-prepared)
    xT = nc.dram_tensor("xT", [D, S], BF, kind="ExternalInput")          # x[batch].T
    # wqk[dt][:, (2h+qk)*128 : +128] = w_qkv d-tile dt for local head h
    wqk = nc.dram_tensor("wqk", [DT, 128, 2 * HLOC * 128], BF,
                         kind="ExternalInput")
    wv = nc.dram_tensor("wv", [DT, 128, CW], BF, kind="ExternalInput")
    bqk = nc.dram_tensor("bqk", [128, 2 * HLOC], F32, kind="ExternalInput")
    bv = nc.dram_tensor("bv", [1, CW], F32, kind="ExternalInput")
    # w_out rows permuted: wout[h][i] = w_out[512*i + 128*h : +128, cols]
    wout = nc.dram_tensor("wout", [HLOC, 4, 128, CW], BF, kind="ExternalInput")
    bout = nc.dram_tensor("bout", [1, CW], F32, kind="ExternalInput")
    out = nc.dram_tensor("out", [S, CW], BF, kind="ExternalOutput")

    # AG buffers: heads 0-2 per token half; head 3 per token quarter
    # (q-block), so the last head's gathers land before its attention ends.
    ag_in = {(h, hf): nc.dram_tensor(f"ag_in{h}_{hf}", [128, 1024], BF,
                                     kind="Internal")
             for h in range(HLAST) for hf in range(2)}
    ag_out = {(h, hf): nc.dram_tensor(f"ag_out{h}_{hf}", [512, 1024], BF,
                                      kind="Internal")
              for h in range(HLAST) for hf in range(2)}
    ag3_in = {qb: nc.dram_tensor(f"ag3_in{qb}", [128, 512], BF, kind="Internal")
              for qb in range(QB)}
    ag3_out = {qb: nc.dram_tensor(f"ag3_out{qb}", [512, 512], BF,
                                  kind="Internal")
               for qb in range(QB)}

    last_attention_mm = [None]   # last PE instruction of head-3 attention
    last_agin_dma = [None]       # last ag_in write on the sync DMA queue

    with tile.TileContext(nc) as tc:
        with (
            tc.tile_pool(name="const", bufs=1) as constp,
            tc.tile_pool(name="pers", bufs=1) as pers,
            tc.tile_pool(name="work", bufs=2) as work,
            tc.tile_pool(name="psum", bufs=2, space="PSUM") as psum,
        ):
            # ---- packed q/k biases: one [128, 8] tile, column h*2+qk ----
            bqk_sb = constp.tile([128, 2 * HLOC], F32, name="bqk_sb")
            nc.sync.dma_start(bqk_sb[:], bqk[:])

            # ---- q/k weight panels: [128, 1024] per d-tile, all heads.
            # Head 0's 256 columns ride a separate first DMA so the first
            # projection chain is not gated on the full panel. ----
            wqk_sb = []
            for dt in range(DT):
                t = constp.tile([128, 2 * HLOC * 128], BF, name=f"wqk{dt}")
                nc.sync.dma_start(t[:, 0:256], wqk[dt, :, 0:256])
                wqk_sb.append(t)

            # xT ships as [128, 1024] half-rows (half the DMA packets of
            # [128, 512] tiles); token half 0 first, then wv, then half 1
            xt_rows = {}
            wv_sb = [None] * DT

            def load_xt(dt, th):
                t = work.tile([128, 1024], BF, name=f"xt_{dt}_{th}",
                              tag="xT", bufs=32)
                nc.sync.dma_start(
                    t[:], xT[dt * 128:(dt + 1) * 128,
                             th * 1024:(th + 1) * 1024])
                xt_rows[(dt, th)] = t

            def xt_ap(dt, tb, lo=0, n=512):
                base = (tb % 2) * 512 + lo
                return xt_rows[(dt, tb // 2)][:, base:base + n]

            for dt in range(DT):
                load_xt(dt, 0)
                wvp = work.tile([128, CW], BF, name=f"wvp{dt}", tag="p512",
                                bufs=17)
                nc.sync.dma_start(wvp[:], wv[dt])
                wv_sb[dt] = wvp
            # heads 1-3 weight columns
            for dt in range(DT):
                nc.sync.dma_start(wqk_sb[dt][:, 256:1024],
                                  wqk[dt, :, 256:1024])
            for dt in range(DT):
                load_xt(dt, 1)

            # ---- per-head q/k projection ([col, tok] transposed) ----
            def qk_proj(h):
                dests = {}
                for qk in range(2):
                    c = (2 * h + qk) * 128
                    dest = work.tile([128, S], BF, name=f"qkT_{h}_{qk}",
                                     tag="qkT", bufs=4)
                    for tb in range(TB):
                        acc = psum.tile([128, 512], F32, name="acc_qk",
                                        tag="acc", bufs=2)
                        for dt in range(DT):
                            nc.tensor.matmul(
                                acc[:], wqk_sb[dt][:, c:c + 128],
                                xt_ap(dt, tb),
                                start=(dt == 0), stop=(dt == DT - 1),
                            )
                        cb = 2 * h + qk
                        nc.scalar.activation(
                            dest[:, tb * 512:(tb + 1) * 512], acc[:],
                            mybir.ActivationFunctionType.Identity,
                            bias=bqk_sb[:, cb:cb + 1], scale=1.0,
                        )
                    dests[qk] = dest
                return dests[0], dests[1]

            qk_tiles = qk_proj(0)

            # ---- constants ----
            ones = constp.tile([128, 1], BF, name="ones")
            nc.gpsimd.memset(ones[:], 1.0)
            ones1 = constp.tile([1, 128], BF, name="ones1")
            nc.gpsimd.memset(ones1[:], 1.0)

            # Pair masks for the 4 diagonal k-subtiles, packed two subtiles
            # wide: pairmask[m][:, 512*sub + qq] keeps where
            # qq >= kk + 128*(2m+sub).
            pairmasks = []
            for pm in range(2):
                m = constp.tile([128, 1024], BF, name=f"pmask{pm}",
                                tag=f"pmask{pm}")
                nc.gpsimd.memset(m[:], 1.0)
                for sub in range(2):
                    nc.gpsimd.affine_select(
                        out=m[:, sub * 512:(sub + 1) * 512],
                        in_=m[:, sub * 512:(sub + 1) * 512],
                        compare_op=mybir.AluOpType.is_ge, fill=0.0,
                        base=-128 * (2 * pm + sub), channel_multiplier=-1,
                        pattern=[[1, 512]],
                    )
                pairmasks.append(m)

            bout_sb = constp.tile([1, CW], F32, name="bout_sb")
            nc.sync.dma_start(bout_sb[:], bout[:])
            bias_bc = constp.tile([128, CW], F32, name="bias_bc")
            nc.gpsimd.partition_broadcast(bias_bc[:], bout_sb[:], channels=128)

            bv_sb = constp.tile([1, CW], F32, name="bv_sb")
            nc.sync.dma_start(bv_sb[:], bv[:])
            vbias_bc = constp.tile([128, CW], F32, name="vbias_bc")
            nc.gpsimd.partition_broadcast(vbias_bc[:], bv_sb[:], channels=128)

            # ---- persistent v tiles ----
            vt = [pers.tile([128, CW], BF, name=f"v{t}", tag=f"v{t}")
                  for t in range(16)]

            # ---- v projection: v[t] = x @ wv  ([tok, vcol], xT stationary) ----
            for t in range(16):
                tb, j = t // 4, t % 4
                acc = psum.tile([128, CW], F32, name="acc_v", tag="acc", bufs=2)
                for dt in range(DT):
                    nc.tensor.matmul(
                        acc[:],
                        xt_ap(dt, tb, j * 128, 128),
                        wv_sb[dt][:],
                        start=(dt == 0), stop=(dt == DT - 1),
                    )
                nc.vector.tensor_tensor(vt[t][:], acc[:], vbias_bc[:],
                                        mybir.AluOpType.add)

            # ---- attention for one head + its AGs.  Heads 0-2 descend
            # through q-blocks (token-half 1 gathered first); head 3
            # ascends and gathers per q-block. ----
            def attention_head(h, qTh, kTh, post_qb=None):
                qb_order = (0, 1, 2, 3) if h == HLAST else (3, 2, 1, 0)
                for qb in qb_order:
                    nk = 4 * qb + 4
                    y_ps = psum.tile([128, 512], F32, name="y_ps", tag="y")
                    esum = work.tile([128, 1024], BF, name="esum", tag="esum",
                                     bufs=2)
                    prev = None

                    def flush(prev_pair):
                        e, pr = prev_pair
                        for s_ in range(2):
                            kt = 2 * pr + s_
                            nc.tensor.matmul(
                                y_ps[:],
                                vt[kt][:, h * 128:(h + 1) * 128],
                                e[:, s_ * 512:(s_ + 1) * 512],
                                start=(kt == 0), stop=(kt == nk - 1),
                            )
                        if pr == 0:
                            nc.vector.tensor_copy(esum[:], e[:])
                        else:
                            nc.vector.tensor_tensor(esum[:], esum[:], e[:],
                                                    mybir.AluOpType.add)

                    for pr in range(nk // 2):
                        sc = psum.tile([128, 1024], F32, name="sc", tag="s",
                                       bufs=2)
                        for s_ in range(2):
                            kt = 2 * pr + s_
                            nc.tensor.matmul(
                                sc[:, s_ * 512:(s_ + 1) * 512],
                                kTh[:, kt * 128:(kt + 1) * 128],
                                qTh[:, qb * 512:(qb + 1) * 512],
                                start=True, stop=True,
                            )
                        e = work.tile([128, 1024], BF, name="expT", tag="expT",
                                      bufs=3)
                        nc.scalar.activation(
                            e[:], sc[:], mybir.ActivationFunctionType.Exp,
                            scale=SCALE,
                        )
                        pm = pr - (nk // 2 - 2)
                        if pm >= 0:
                            nc.vector.tensor_tensor(e[:], e[:],
                                                    pairmasks[pm][:],
                                                    mybir.AluOpType.mult)
                        if prev is not None:
                            flush(prev)
                        prev = (e, pr)
                    flush(prev)

                    esum_f = work.tile([128, 512], BF, name="esum_f",
                                       tag="esum_f", bufs=2)
                    nc.vector.tensor_tensor(esum_f[:], esum[:, 0:512],
                                            esum[:, 512:1024],
                                            mybir.AluOpType.add)
                    # nb: one PSUM bank serving the partition-sum (row 0)
                    # and then the broadcast of its reciprocal (all rows).
                    nb = psum.tile([128, 512], F32, name="nb", tag="y")
                    nc.tensor.matmul(nb[0:1, :], ones[:], esum_f[:],
                                     start=True, stop=True)
                    recip = work.tile([1, 512], F32, name="recip", tag="recip",
                                      bufs=2)
                    nc.vector.reciprocal_approx_fast(recip[:], nb[0:1, :])
                    recip_b = work.tile([1, 512], BF, name="recip_b",
                                        tag="recip_b", bufs=2)
                    nc.vector.tensor_copy(recip_b[:], recip[:])
                    mm_bc = nc.tensor.matmul(nb[:, :], ones1[:], recip_b[:],
                                             start=True, stop=True)
                    if h == HLAST:
                        last_attention_mm[0] = mm_bc
                    # DVE can't read two PSUM operands; stage the broadcast
                    # through SBUF on the (otherwise idle) scalar engine.
                    nbs = work.tile([128, 512], BF, name="nbs", tag="nbs",
                                    bufs=2)
                    nc.scalar.activation(
                        nbs[:], nb[:], mybir.ActivationFunctionType.Identity,
                        scale=1.0,
                    )
                    ynorm = work.tile([128, 512], BF, name="ynorm", tag="ynorm",
                                      bufs=3)
                    nc.vector.tensor_tensor(ynorm[:], y_ps[:], nbs[:],
                                            mybir.AluOpType.mult)
                    if h == HLAST:
                        d = nc.sync.dma_start(ag3_in[qb][:], ynorm[:])
                        last_agin_dma[0] = d
                        nc.gpsimd.collective_compute(
                            "AllGather", mybir.AluOpType.bypass,
                            replica_groups=GROUPS,
                            ins=[ag3_in[qb].ap()],
                            outs=[ag3_out[qb].ap()],
                        )
                        if post_qb is not None:
                            post_qb(qb)
                    else:
                        hf, co = qb // 2, (qb % 2) * 512
                        nc.sync.dma_start(
                            ag_in[(h, hf)][:, co:co + 512], ynorm[:])
                        if qb in (2, 0):
                            nc.gpsimd.collective_compute(
                                "AllGather", mybir.AluOpType.bypass,
                                replica_groups=GROUPS,
                                ins=[ag_in[(h, hf)].ap()],
                                outs=[ag_out[(h, hf)].ap()],
                            )

            # ---- head pipeline.  During head 3's attention, each q-block
            # hook issues the ygt readbacks for one out-projection block
            # (their AG waits are already satisfied, so the sync DMA queue
            # never head-of-line blocks between head-3's ag writes). ----
            wout_sb = {}
            ygt = {}
            ygt3 = {}

            def load_wout():
                for h in range(HLOC):
                    for i in range(4):
                        t = work.tile([128, CW], BF, name=f"wout{h}{i}",
                                      tag="p512", bufs=17)
                        nc.sync.dma_start(t[:], wout[h, i])
                        wout_sb[(h, i)] = t

            def load_ygt(tc_):
                hf, co = tc_ // 2, (tc_ % 2) * 512
                for h in range(HLAST):
                    for i in range(4):
                        t = work.tile([128, 512], BF, name=f"yg_{h}_{tc_}_{i}",
                                      tag="ygt", bufs=16)
                        nc.sync.dma_start(
                            t[:], ag_out[(h, hf)][i * 128:(i + 1) * 128,
                                                  co:co + 512])
                        ygt[(h, tc_, i)] = t

            def load_ygt3():
                for tc_ in (0, 1, 2, 3):
                    for i in range(4):
                        t3 = work.tile([128, 512], BF, name=f"yg3_{tc_}_{i}",
                                       tag="ygt3", bufs=6)
                        nc.sync.dma_start(
                            t3[:], ag3_out[tc_][i * 128:(i + 1) * 128, :])
                        ygt3[(tc_, i)] = t3

            def att3_hook(qb):
                load_ygt({0: 2, 1: 3, 2: 0, 3: 1}[qb])
                if qb == 3:
                    load_ygt3()

            for h in range(HLOC):
                if h == HLAST:
                    attention_head(h, *qk_tiles, post_qb=att3_hook)
                else:
                    attention_head(h, *qk_tiles)
                    qk_tiles = qk_proj(h + 1)
                    if h == 0:
                        load_wout()

            # ---- out projection, split per token tile:
            #   chain A: heads 0-2 (12 matmuls) -> bf16 partial (+bias) in
            #            SBUF slots the xT pool no longer needs.  This is
            #            the PE filler during head-3's attention.
            #   chain B: head 3 (4 matmuls), ordered after the last
            #            attention matmul, + final combine and store. ----
            first_b_mm = [None]
            partA = {}
            for tc_ in (2, 3, 0, 1):
                for j in range(4):
                    t = tc_ * 4 + j
                    accA = psum.tile([128, CW], F32, name="acc_a",
                                     tag="acc", bufs=2)
                    nmm = 0
                    for h in range(HLAST):
                        for i in range(4):
                            nc.tensor.matmul(
                                accA[:],
                                ygt[(h, tc_, i)][:, j * 128:(j + 1) * 128],
                                wout_sb[(h, i)][:],
                                start=(nmm == 0), stop=(nmm == 11),
                            )
                            nmm += 1
                    if t % 2 == 0:
                        partA[t // 2] = work.tile([128, 1024], BF,
                                                  name=f"partA{t // 2}",
                                                  tag="xT", bufs=32)
                    pa = partA[t // 2][:, (t % 2) * 512:(t % 2) * 512 + 512]
                    nc.vector.tensor_tensor(pa, accA[:], bias_bc[:],
                                            mybir.AluOpType.add)
            for tc_ in (0, 1, 2, 3):
                for j in range(4):
                    t = tc_ * 4 + j
                    accB = psum.tile([128, CW], F32, name="acc_b",
                                     tag="y", bufs=2)
                    for i in range(4):
                        mm = nc.tensor.matmul(
                            accB[:],
                            ygt3[(tc_, i)][:, j * 128:(j + 1) * 128],
                            wout_sb[(HLAST, i)][:],
                            start=(i == 0), stop=(i == 3),
                        )
                        if first_b_mm[0] is None:
                            first_b_mm[0] = mm
                    osb = work.tile([128, CW], BF, name="osb",
                                    tag="osb", bufs=3)
                    pa = partA[t // 2][:, (t % 2) * 512:(t % 2) * 512 + 512]
                    nc.vector.tensor_tensor(osb[:], accB[:], pa,
                                            mybir.AluOpType.add)
                    nc.sync.dma_start(out[t * 128:t * 128 + 64, :],
                                      osb[0:64, :])
                    nc.sync.dma_start(out[t * 128 + 64:(t + 1) * 128, :],
                                      osb[64:128, :])

            add_dep_helper(first_b_mm[0].ins, last_attention_mm[0].ins,
                           sync=False,
                           reason="h3 outproj PE stream after last attention mm")

    nc.compile()
    return nc


def _prep_inputs(x, w_qkv, b_qkv, w_out, b_out):
    """Host-side sharding/layout. Returns in_maps for the 8 cores."""
    bf16 = ml_dtypes.bfloat16
    x = np.asarray(x, dtype=np.float32)
    w_qkv = np.asarray(w_qkv, dtype=np.float32)
    b_qkv = np.asarray(b_qkv, dtype=np.float32)
    w_out = np.asarray(w_out, dtype=np.float32)
    b_out = np.asarray(b_out, dtype=np.float32)

    xT_b = [np.ascontiguousarray(x[b].T).astype(bf16) for b in range(B)]

    in_maps = []
    for c in range(8):
        b, g = c // 4, c % 4
        cols = slice(CW * g, CW * (g + 1))

        # wqk[dt][:, (2h+qk)*128:+128] = d-tile dt of w_q/w_k for head 4g+h
        wqk = np.empty((DT, 128, 2 * HLOC * 128), np.float32)
        bqk = np.empty((128, 2 * HLOC), np.float32)
        for h in range(HLOC):
            gh = 4 * g + h
            for qk in range(2):
                wcol = w_qkv[:, qk * D + 128 * gh: qk * D + 128 * (gh + 1)]
                wqk[:, :, (2 * h + qk) * 128:(2 * h + qk + 1) * 128] = \
                    wcol.reshape(DT, 128, 128)
                bqk[:, 2 * h + qk] = b_qkv[qk * D + 128 * gh: qk * D + 128 * (gh + 1)]

        wv_ = w_qkv[:, 2 * D:3 * D][:, cols]
        bv_ = b_qkv[2 * D:3 * D][cols]

        # w_out rows permuted to the AG's rank-major order per head chunk
        wout_loc = w_out[:, cols]
        wout_t = np.empty((HLOC, 4, 128, CW), np.float32)
        for h in range(HLOC):
            for i in range(4):
                wout_t[h, i] = wout_loc[512 * i + 128 * h: 512 * i + 128 * (h + 1), :]

        in_maps.append({
            "xT": xT_b[b],
            "wqk": np.ascontiguousarray(wqk).astype(bf16),
            "wv": np.ascontiguousarray(wv_.reshape(DT, 128, CW)).astype(bf16),
            "bqk": np.ascontiguousarray(bqk),
            "bv": np.ascontiguousarray(bv_.reshape(1, CW)),
            "wout": np.ascontiguousarray(wout_t).astype(bf16),
            "bout": np.ascontiguousarray(b_out[cols].reshape(1, CW)),
        })
    return in_maps


def kernel(x, w_qkv, b_qkv, w_out, b_out, _trace=False, _trace_kwargs=None):
    from concourse.bass_utils import run_bass_kernel_spmd

    if "nc" not in _cache:
        _cache["nc"] = _build()
    nc = _cache["nc"]

    in_maps = _prep_inputs(x, w_qkv, b_qkv, w_out, b_out)
    res = run_bass_kernel_spmd(
        nc, in_maps, core_ids=list(range(8)),
        trace=_trace, **(_trace_kwargs or {}),
    )

    out = np.empty((B, S, D), dtype=np.float32)
    for c in range(8):
        b, g = c // 4, c % 4
        out[b][:, CW * g:CW * (g + 1)] = np.asarray(res.results[c]["out"], dtype=np.float32)
    kernel.last_result = res
    return out
